# revision 1
# baseline (speedup 1.0000x reference)
"""ALSHConvNet forward on 8 TRN2 NeuronCores (Bass/Tile), batch-sharded.

Host side: shards batch 2048 -> 8 x 256, precomputes weight/staging layouts,
layer-1 mask (from x channel means, exact reference math), and hash constants.
Device side (per core): 3 convs as tiled bf16 matmuls with kx PSUM-accumulation,
maxpool fused into PSUM evacuation (x) and parity-split DMA + TT max (y),
relu+mask+channel-sums in one ACT op, device hash chains for masks 2/3 with two
16/20-float AllReduces, FC with mask-3 folded into Wout.
"""
import sys
sys.path.insert(0, '/opt/trn_rl_repo')
import numpy as np
import ml_dtypes

import concourse.bacc as bacc
import concourse.mybir as mybir
import concourse.tile as tile
from concourse.bass_utils import run_bass_kernel_spmd

N_CORES = 8
B_LOC = 256
fp32 = mybir.dt.float32
bf16 = mybir.dt.bfloat16
AL = mybir.AluOpType
AF = mybir.ActivationFunctionType
AX = mybir.AxisListType
BF = ml_dtypes.bfloat16

U = 0.9999
M_APPEND = 3
TABLE = 25

# fraction of PSUM-evac tiles handled by the ACT(copy)+DVE(xpool) path;
# remainder by the DVE(reduce_max) direct path
ACT_EVAC_FRAC = 0.25


# ---------------------------------------------------------------- host math
def _filter_codes(W, A):
    out_ch = W.shape[0]
    Wf = np.asarray(W, np.float32).reshape(out_ch, -1)
    norms = np.sqrt((Wf * Wf).sum(1))
    Wp = Wf * (U / norms.max())
    n2 = (Wp * Wp).sum(1)
    terms = np.stack([0.5 - n2 ** (2 ** i) for i in range(M_APPEND)], 1)
    P = np.concatenate([Wp, terms], 1).astype(np.float32)
    zW = np.einsum('lkd,nd->lkn', np.asarray(A, np.float32), P)
    K = A.shape[1]
    bits = (2 ** np.arange(K)).astype(np.int64)
    return (((zW > 0).astype(np.int64)) * bits[None, :, None]).sum(1) % TABLE


def _query_mask(cm, W, A):
    codeW = _filter_codes(W, A)
    kh = W.shape[2] * W.shape[3]
    q = np.tile(np.asarray(cm, np.float32)[:, None], (1, kh)).reshape(-1)
    q = q / (np.sqrt((q * q).sum()) + 1e-8)
    Qv = np.concatenate([q, np.zeros(M_APPEND, np.float32)])
    zQ = np.einsum('lkd,d->lk', np.asarray(A, np.float32), Qv)
    K = A.shape[1]
    bits = (2 ** np.arange(K)).astype(np.int64)
    codeQ = (((zQ > 0).astype(np.int64)) * bits[None, :]).sum(1) % TABLE
    return (codeW == codeQ[:, None]).any(0)


def _collapse_A(A, in_ch):
    """A [L,K,in_ch*25+3] -> Atil [L,K,in_ch]: sum over the 25 kernel reps."""
    A = np.asarray(A, np.float32)
    return A[:, :, :in_ch * 25].reshape(A.shape[0], A.shape[1], in_ch, 25).sum(3)


def _host_prep(x, W1, A1, W2, A2, W3, A3, Wout, bout, force_masks=False):
    """Build all per-core / shared device tensors. Returns (shared, per_core list)."""
    x = np.asarray(x, np.float32)
    W1 = np.asarray(W1, np.float32)
    W2 = np.asarray(W2, np.float32)
    W3 = np.asarray(W3, np.float32)
    Wout = np.asarray(Wout, np.float32)
    bout = np.asarray(bout, np.float32)

    # ---- layer-1 mask from full-batch channel means (host) ----
    m1 = _query_mask(x.mean((0, 2, 3)), W1, A1).astype(np.float32)  # (16,)
    if force_masks:
        m1 = np.ones(16, np.float32)

    # ---- X staging: [128, 2*256*36] bf16 ----
    # quadrant r rows (iy8, ic3) iy-major; windows w = c*4 + r at wsel=c
    xs_all = []
    xsh = x.reshape(N_CORES, B_LOC, 3, 32, 32)
    xpad = np.zeros((N_CORES, B_LOC, 3, 36, 36), np.float32)
    xpad[:, :, :, 2:34, 2:34] = xsh
    for core in range(N_CORES):
        X4 = np.zeros((128, 2 * B_LOC * 36), np.float32)
        for w in range(8):
            r, c = w % 4, w // 4
            for iy in range(8):
                gy = 4 * w - 2 + iy + 2  # row in padded (0..35)
                for ic in range(3):
                    row = 32 * r + iy * 3 + ic
                    X4[row, c * (B_LOC * 36):(c + 1) * (B_LOC * 36)] = \
                        xpad[core, :, ic, gy, :].reshape(-1)
        xs_all.append(X4.astype(BF))

    # ---- weight stagings ----
    # L1: W1s [128, 5*64]: quadrant r rows (iy8,ic3); col (s*64 + par*32+oyh*16+oc)
    W1s = np.zeros((128, 5 * 64), np.float32)
    for r in range(4):
        for iy in range(8):
            for ic in range(3):
                row = 32 * r + iy * 3 + ic
                for s in range(5):
                    for par in range(2):
                        for oyh in range(2):
                            oy = 2 * oyh + par
                            ky = iy - oy
                            if 0 <= ky < 5:
                                for oc in range(16):
                                    W1s[row, s * 64 + par * 32 + oyh * 16 + oc] = \
                                        W1[oc, ic, ky, s]
    # L2: W2s [128, 5*80]: rows (iy8, ic16) iy-major; lane par*40+oyh*20+oc
    W2s = np.zeros((128, 5 * 80), np.float32)
    for iy in range(8):
        for ic in range(16):
            row = iy * 16 + ic
            for s in range(5):
                for par in range(2):
                    for oyh in range(2):
                        oy = 2 * oyh + par
                        ky = iy - oy
                        if 0 <= ky < 5:
                            for oc in range(20):
                                W2s[row, s * 80 + par * 40 + oyh * 20 + oc] = \
                                    W2[oc, ic, ky, s]
    # L3: W3s [120, 5*40]: rows (iy6, ic20) iy-major; lane par*20+oc
    W3s = np.zeros((120, 5 * 40), np.float32)
    for iy in range(6):
        for ic in range(20):
            row = iy * 20 + ic
            for s in range(5):
                for par in range(2):
                    ky = iy - par
                    if 0 <= ky < 5:
                        for oc in range(20):
                            W3s[row, s * 40 + par * 20 + oc] = W3[oc, ic, ky, s]
    # FC: Wouts [80, 4*10]: rows (y'4, oc20) = y'*20+oc; col x'*10+co
    Wouts = np.zeros((80, 40), np.float32)
    for oc in range(20):
        for yq in range(4):
            for xq in range(4):
                for co in range(10):
                    Wouts[yq * 20 + oc, xq * 10 + co] = Wout[co, oc * 16 + yq * 4 + xq]

    # ---- hash constants ----
    At2 = _collapse_A(A2, 16) * m1[None, None, :]       # (2,6,16), m1 folded
    A2T = At2.transpose(2, 0, 1).reshape(16, 12).copy() # rows c, cols (l,k)
    At3 = _collapse_A(A3, 20)                           # (3,10,20)
    A3T = At3.transpose(2, 0, 1).reshape(20, 30).copy()
    BW2 = np.zeros((12, 2), np.float32)
    for l in range(2):
        for k in range(6):
            BW2[l * 6 + k, l] = float((2 ** k) % TABLE)
    BW3 = np.zeros((30, 3), np.float32)
    for l in range(3):
        for k in range(10):
            BW3[l * 10 + k, l] = float((2 ** k) % TABLE)
    CW2 = _filter_codes(W2, A2).astype(np.float32)      # (2,20)
    CW3 = _filter_codes(W3, A3).astype(np.float32)      # (3,20)
    ONES2 = np.ones((2, 1), np.float32)
    ONES3 = np.ones((3, 1), np.float32)
    # P1 lanes: r4*32 + oyh2*16 + oc16 -> oc = lane % 16
    IND1 = np.zeros((128, 16), np.float32)
    for lane in range(128):
        IND1[lane, lane % 16] = 1.0
    # Q2 lanes: q2*40 + oyh2*20 + oc20 -> oc = lane % 20
    IND2 = np.zeros((80, 20), np.float32)
    for lane in range(80):
        IND2[lane, lane % 20] = 1.0
    # EXP2 [20,80]: m[20] -> 80 lanes with oc = lane % 20 (Q2 and P3 layouts)
    EXP2 = np.zeros((20, 80), np.float32)
    for lane in range(80):
        EXP2[lane % 20, lane] = 1.0
    # m1 expanded to P1 lanes (oc = lane % 16)
    m1exp = np.tile(m1, 8)[:, None].astype(np.float32)  # (128,1)
    # mean scales folded: reference cm2 = sum(h1)/(2048*16*16); A2T already has m1.
    # sign-invariant to positive scale -> no normalization needed.
    boutc = bout.reshape(10, 1).astype(np.float32)

    if force_masks:
        BW2[:] = 0.0; BW3[:] = 0.0; CW2 = np.zeros_like(CW2); CW3 = np.zeros_like(CW3)
    shared = dict(
        W1s=W1s.astype(BF), W2s=W2s.astype(BF), W3s=W3s.astype(BF),
        Wouts=Wouts.astype(BF),
        A2T=A2T, A3T=A3T, BW2=BW2, BW3=BW3, CW2=CW2, CW3=CW3,
        ONES2=ONES2, ONES3=ONES3, IND1=IND1, IND2=IND2, EXP2=EXP2,
        m1exp=m1exp, boutc=boutc,
        zeros=np.zeros((128, 5120), BF),
    )
    return shared, xs_all


# ---------------------------------------------------------------- device build
def build_nc(debug=False):
    nc = bacc.Bacc("TRN2", target_bir_lowering=False, debug=False,
                   num_devices=N_CORES)

    ext = {}
    def ein(name, shape, dt):
        ext[name] = nc.dram_tensor(name, shape, dt, kind="ExternalInput")
        return ext[name]

    ein("X4", [128, 2 * B_LOC * 36], bf16)
    ein("W1s", [128, 320], bf16)
    ein("W2s", [128, 400], bf16)
    ein("W3s", [120, 200], bf16)
    ein("Wouts", [80, 40], bf16)
    ein("A2T", [16, 12], fp32)
    ein("A3T", [20, 30], fp32)
    ein("BW2", [12, 2], fp32)
    ein("BW3", [30, 3], fp32)
    ein("CW2", [2, 20], fp32)
    ein("CW3", [3, 20], fp32)
    ein("ONES2", [2, 1], fp32)
    ein("ONES3", [3, 1], fp32)
    ein("IND1", [128, 16], fp32)
    ein("IND2", [80, 20], fp32)
    ein("EXP2", [20, 80], fp32)
    ein("m1exp", [128, 1], fp32)
    ein("boutc", [10, 1], fp32)
    ein("zeros", [128, 5120], bf16)

    out_ext = nc.dram_tensor("out", [10, B_LOC], fp32, kind="ExternalOutput")
    dbg = {}
    if debug:
        for nm, shape in [("d_XP1_0", [128, 4096]), ("d_P1_0", [128, 4096]),
                          ("d_P1_1", [128, 4096]),
                           ("d_XP2_0", [128, 2048]),
                          ("d_Q2_0", [80, 2048]), ("d_Q2_1", [80, 2048]),
                          ("d_S3_1", [120, 3072]),
                          ("d_XP3_0", [128, 1024]), ("d_P3", [80, 1024]),
                          ("d_m", [80, 4])]:
            dbg[nm] = nc.dram_tensor(nm, shape, fp32, kind="ExternalOutput")

    with tile.TileContext(nc, num_cores=N_CORES) as tc:
        with (
            tc.tile_pool(name="const", bufs=1) as cpool,
            tc.tile_pool(name="work", bufs=1) as wpool,
            tc.tile_pool(name="scratch", bufs=(3 if debug else 6)) as spool,
            tc.tile_pool(name="dbgpool", bufs=1) as dbgpool,
            tc.tile_pool(name="psum", bufs=8, space="PSUM") as pp,
            tc.tile_pool(name="dram", bufs=1, space="DRAM") as dpool,
        ):
            # ------- X4 load first (largest, gates L1) -------
            X4g = wpool.tile([128, 2 * B_LOC * 36], bf16, tag="X4",
                             name="X4")
            nc.sync.dma_start(X4g[:], ext["X4"].ap())

            # ------- load constants (gpsimd queue; sync stays free) -------
            def load(name, shape, dt, pool=cpool):
                t = pool.tile(shape, dt, tag=name)
                nc.gpsimd.dma_start(t[:], ext[name].ap())
                return t

            W1sb = load("W1s", [128, 320], bf16)
            W2sb = load("W2s", [128, 400], bf16)
            W3sb = load("W3s", [120, 200], bf16)
            Woutsb = load("Wouts", [80, 40], bf16)
            A2Tb = load("A2T", [16, 12], fp32)
            A3Tb = load("A3T", [20, 30], fp32)
            BW2b = load("BW2", [12, 2], fp32)
            BW3b = load("BW3", [30, 3], fp32)
            CW2b = load("CW2", [2, 20], fp32)
            CW3b = load("CW3", [3, 20], fp32)
            ONES2b = load("ONES2", [2, 1], fp32)
            ONES3b = load("ONES3", [3, 1], fp32)
            IND1b = load("IND1", [128, 16], fp32)
            IND2b = load("IND2", [80, 20], fp32)
            EXP2b = load("EXP2", [20, 80], fp32)
            m1expb = load("m1exp", [128, 1], fp32)
            boutb = load("boutc", [10, 1], fp32)
            one1 = cpool.tile([1, 1], fp32, tag="one1", name="one1")
            nc.vector.memset(one1[:], 1.0)

            # PE warm-up burst (overlaps the X4 DMA; warms the HAM clock)
            for _ in range(3):
                wps = pp.tile([128, 320], fp32, tag="convps", name="convps")
                for ws in range(5):
                    nc.tensor.matmul(wps[:], W1sb[:, 0:128], W1sb[:],
                                     start=(ws == 0), stop=(ws == 4))

            zext = ext["zeros"].ap()

            def dump(nm, t):
                if not debug:
                    return
                f = dbgpool.tile(list(t.shape), bf16, tag="dumpf", name="dumpf")
                nc.vector.tensor_copy(f[:], t[:])
                nc.gpsimd.dma_start(dbg[nm].ap(), f[:])

            # persistent pooled-activation tiles (outer pool)
            P1 = [wpool.tile([128, 16 * 256], bf16, tag=f"P1_{c}", name=f"P1_{c}")
                  for c in range(2)]
            S1acc = wpool.tile([128, 2], fp32, tag="S1acc", name="S1acc")
            Q2 = [wpool.tile([80, 8 * 256], bf16, tag=f"Q2_{h}", name=f"Q2_{h}") for h in range(2)]
            S2 = [wpool.tile([128, 20 * B_LOC], bf16, tag=f"S2_{w}",
                             name="S2") for w in range(4)]
            S3 = [wpool.tile([120, 12 * B_LOC], bf16, tag=f"S3_{w}",
                             name="S3") for w in range(4)]
            # pad DMAs issued up-front (depend on nothing)
            for w in range(4):
                dpad = S2[w].rearrange("p (blk q) -> p blk q", q=512)
                nc.sync.dma_start(dpad[:, 0:10:9, :],
                                  zext[0:128, 0:1024].rearrange(
                                      "p (a q) -> p a q", q=512))
                dpad3 = S3[w].rearrange("p (blk q) -> p blk q", q=512)
                nc.sync.dma_start(dpad3[:, 0:6:5, :],
                                  zext[0:120, 0:1024].rearrange(
                                      "p (a q) -> p a q", q=512))
            nc.sync.dma_start(S2[0][0:32, 2 * B_LOC:18 * B_LOC],
                              zext[0:32, 0:16 * B_LOC])
            nc.sync.dma_start(S2[3][96:128, 2 * B_LOC:18 * B_LOC],
                              zext[0:32, 0:16 * B_LOC])
            nc.sync.dma_start(S3[0][0:40, 2 * B_LOC:10 * B_LOC],
                              zext[0:40, 0:8 * B_LOC])
            nc.sync.dma_start(S3[3][80:120, 2 * B_LOC:10 * B_LOC],
                              zext[0:40, 0:8 * B_LOC])
            S2acc = wpool.tile([80, 2], fp32, tag="S2acc", name="S2acc")
            P3 = wpool.tile([80, 4 * 256], bf16, tag="P3", name="P3")

            # ======================= LAYER 1 =======================
            with tc.tile_pool(name="l1big", bufs=1) as l1pool:
                X4 = X4g
                XP1 = [l1pool.tile([128, 16 * 256], bf16, tag=f"XP1_{r}", name=f"XP1_{r}")
                       for r in range(4)]

                tile_count = 0
                v = X4.rearrange("p (w b x) -> p w b x", w=2, b=B_LOC)
                for j in range(16):
                    pss = [pp.tile([128, 512], fp32, tag="convps",
                                   name="convps") for _ in range(4)]
                    for s in range(5):
                        for c in range(2):
                            for r in range(4):
                                nc.tensor.matmul(
                                    pss[r][64 * c:64 * c + 64, :],
                                    W1sb[32 * r:32 * r + 24,
                                         s * 64:(s + 1) * 64],
                                    v[32 * r:32 * r + 24, c, :,
                                      2 * j + s:2 * j + s + 2],
                                    start=(s == 0), stop=(s == 4),
                                    tile_position=(32 * r, 64 * c))
                    for r in range(4):
                        ps = pss[r]
                        use_act = (tile_count % 20) < int(ACT_EVAC_FRAC * 20)
                        tile_count += 1
                        if use_act:
                            sc = spool.tile([128, 512], bf16, tag="evac",
                                            name="evac")
                            nc.scalar.activation(sc[:], ps[:], AF.Copy)
                            vv = sc.rearrange("p (b x) -> p b x", x=2)
                            nc.vector.tensor_tensor(
                                out=XP1[r][:, j * 256:j * 256 + 256],
                                in0=vv[:, :, 0], in1=vv[:, :, 1], op=AL.max)
                        else:
                            nc.vector.tensor_reduce(
                                out=XP1[r][:, j * 256:j * 256 + 256],
                                in_=ps.rearrange("p (b x) -> p b x", x=2),
                                axis=AX.X, op=AL.max)

                # y-pool: parity-split DMAs then TT max -> P1
                P1A = [l1pool.tile([128, 16 * 256], bf16, tag=f"P1A_{c}", name=f"P1A_{c}")
                       for c in range(2)]
                P1B = [l1pool.tile([128, 16 * 256], bf16, tag=f"P1B_{c}", name=f"P1B_{c}")
                       for c in range(2)]
                # P1 lanes: 32r + 16oyh + oc (y' = 8c + 2r + oyh)
                _dq = 0
                for r in range(4):
                    for c in range(2):
                        for par, dstt in ((0, P1A), (1, P1B)):
                            eng = nc.sync if _dq % 2 == 0 else nc.gpsimd
                            _dq += 1
                            eng.dma_start(
                                dstt[c][32 * r:32 * r + 32, :],
                                XP1[r][64 * c + 32 * par:
                                       64 * c + 32 * par + 32, :])
                dump("d_XP1_0", XP1[0])
                for c in range(2):
                    nc.vector.tensor_tensor(out=P1[c][:], in0=P1A[c][:],
                                            in1=P1B[c][:], op=AL.max)
                    nc.scalar.activation(P1[c][:], P1[c][:], AF.Relu,
                                         scale=m1expb[:],
                                         accum_out=S1acc[:, c:c + 1])
                dump("d_P1_0", P1[0])
                dump("d_P1_1", P1[1])

            # ---- s1 partial + AllReduce ----
            s1ps = pp.tile([16, 2], fp32, tag="convps", name="convps")
            nc.tensor.matmul(s1ps[:], IND1b[:], S1acc[:], start=True, stop=True)
            s1loc = cpool.tile([16, 2], fp32, tag="s1loc", name="s1loc")
            nc.vector.tensor_copy(s1loc[:], s1ps[:])
            s1l = cpool.tile([16, 1], fp32, tag="s1l", name="s1l")
            nc.vector.tensor_tensor(out=s1l[:], in0=s1loc[:, 0:1],
                                    in1=s1loc[:, 1:2], op=AL.add)
            cc1_in = dpool.tile([16, 1], fp32)
            cc1_out = dpool.tile([16, 1], fp32)
            nc.sync.dma_start(cc1_in[:], s1l[:])
            nc.gpsimd.collective_compute(
                "AllReduce", AL.add, replica_groups=[list(range(N_CORES))],
                ins=[cc1_in.opt()], outs=[cc1_out.opt()])
            s1f = cpool.tile([16, 1], fp32, tag="s1f", name="s1f")
            nc.sync.dma_start(s1f[:], cc1_out[:])

            # ---- hash chain ----
            def hash_chain(v_sb, ATb, BWb, CWb, ONESb, LK, L, name):
                zps = pp.tile([LK, 1], fp32, tag="convps", name="convps")
                nc.tensor.matmul(zps[:], ATb[:], v_sb[:], start=True, stop=True)
                bits = cpool.tile([LK, 1], fp32, tag=f"bits{name}", name=f"bits{name}")
                nc.vector.tensor_scalar(out=bits[:], in0=zps[:], scalar1=0.0,
                                        scalar2=None, op0=AL.is_gt)
                yps = pp.tile([L, 1], fp32, tag="convps", name="convps")
                nc.tensor.matmul(yps[:], BWb[:], bits[:], start=True, stop=True)
                code = cpool.tile([L, 1], fp32, tag=f"code{name}", name=f"code{name}")
                nc.vector.tensor_copy(code[:], yps[:])
                ge = cpool.tile([L, 1], fp32, tag=f"ge{name}", name=f"ge{name}")
                for _ in range(2):
                    nc.vector.tensor_scalar(out=ge[:], in0=code[:],
                                            scalar1=24.5, scalar2=None,
                                            op0=AL.is_gt)
                    nc.vector.tensor_scalar(out=ge[:], in0=ge[:], scalar1=25.0,
                                            scalar2=None, op0=AL.mult)
                    nc.vector.tensor_tensor(out=code[:], in0=code[:],
                                            in1=ge[:], op=AL.subtract)
                eq = cpool.tile([L, 20], fp32, tag=f"eq{name}", name=f"eq{name}")
                nc.vector.tensor_tensor(out=eq[:], in0=CWb[:],
                                        in1=code[:].broadcast_to((L, 20)),
                                        op=AL.is_equal)
                cps = pp.tile([1, 20], fp32, tag="convps", name="convps")
                nc.tensor.matmul(cps[:], ONESb[:], eq[:], start=True, stop=True)
                mrow = cpool.tile([1, 20], fp32, tag=f"mrow{name}", name=f"mrow{name}")
                nc.vector.tensor_scalar(out=mrow[:], in0=cps[:], scalar1=0.5,
                                        scalar2=None, op0=AL.is_gt)
                mtp = pp.tile([20, 1], fp32, tag="convps", name="convps")
                nc.tensor.transpose(mtp[:], mrow[:], one1[:])
                mcol = cpool.tile([20, 1], fp32, tag=f"mcol{name}", name=f"mcol{name}")
                nc.vector.tensor_copy(mcol[:], mtp[:])
                return mcol

            # ======================= LAYER 2 =======================
            with tc.tile_pool(name="l2big", bufs=1) as l2pool:
                # S2_w [128=(iy8,ic16), (xp20, b256)]
                # S2 valid-row staging (pads already zeroed up-front)
                for w in range(4):
                    y0, y1 = max(0, 4 * w - 2), min(16, 4 * w + 6)
                    iy0 = y0 - (4 * w - 2)
                    for ch in range(2):
                        ya, yb = max(y0, 8 * ch), min(y1, 8 * ch + 8)
                        if ya >= yb:
                            continue
                        la = 16 * (ya % 8)
                        da = 16 * (iy0 + (ya - y0))
                        nc.sync.dma_start(
                            S2[w][da:da + 16 * (yb - ya),
                                  2 * B_LOC:18 * B_LOC],
                            P1[ch][la:la + 16 * (yb - ya), :])

                XP2 = [l2pool.tile([128, 8 * 256], bf16, tag=f"XP2_{w}", name=f"XP2_{w}")
                       for w in range(4)]
                tile_count = 0
                for w in range(4):
                    rhsv = S2[w].rearrange("p (xp b) -> p b xp", b=B_LOC)
                    for jg in range(2):
                        pss = [pp.tile([128, 512], fp32, tag="convps",
                                       name="convps") for _ in range(4)]
                        for srt in range(5):
                            for jj in range(4):
                                j = 4 * jg + jj
                                nc.tensor.matmul(
                                    pss[jj][0:80, :],
                                    W2sb[:, srt * 80:(srt + 1) * 80],
                                    rhsv[:, :, 2 * j + srt:2 * j + srt + 2],
                                    start=(srt == 0), stop=(srt == 4))
                        for jj in range(4):
                            j = 4 * jg + jj
                            ps = pss[jj]
                            use_act = (tile_count % 20) < int(ACT_EVAC_FRAC * 20)
                            tile_count += 1
                            if use_act:
                                sc = spool.tile([128, 512], bf16, tag="evac",
                                                name="evac")
                                nc.scalar.activation(sc[0:80, :], ps[0:80, :],
                                                     AF.Copy)
                                vv = sc.rearrange("p (b x) -> p b x", x=2)
                                nc.vector.tensor_tensor(
                                    out=XP2[w][0:80, j * 256:j * 256 + 256],
                                    in0=vv[0:80, :, 0], in1=vv[0:80, :, 1],
                                    op=AL.max)
                            else:
                                nc.vector.tensor_reduce(
                                    out=XP2[w][0:80, j * 256:j * 256 + 256],
                                    in_=ps[0:80, :].rearrange(
                                        "p (b x) -> p b x", x=2),
                                    axis=AX.X, op=AL.max)

                m2col = hash_chain(s1f, A2Tb, BW2b, CW2b, ONES2b,
                                   12, 2, "m2")
                m2ps = pp.tile([80, 1], fp32, tag="convps", name="convps")
                nc.tensor.matmul(m2ps[:], EXP2b[:], m2col[:],
                                 start=True, stop=True)
                m2exp = cpool.tile([80, 1], fp32, tag="m2exp", name="m2exp")
                nc.vector.tensor_copy(m2exp[:], m2ps[:])

                Q2A = [l2pool.tile([80, 8 * 256], bf16, tag=f"Q2A_{h}",
                                   name="Q2A") for h in range(2)]
                Q2B = [l2pool.tile([80, 8 * 256], bf16, tag=f"Q2B_{h}",
                                   name="Q2B") for h in range(2)]
                # Q2 lanes: 40q + 20oyh + oc (y' = 4h + 2q + oyh, q = w % 2)
                _dq = 0
                for w in range(4):
                    h, q = w // 2, w % 2
                    for par, dstt in ((0, Q2A), (1, Q2B)):
                        eng = nc.sync if _dq % 2 == 0 else nc.gpsimd
                        _dq += 1
                        eng.dma_start(
                            dstt[h][40 * q:40 * q + 40, :],
                            XP2[w][40 * par:40 * par + 40, :])
                dump("d_XP2_0", XP2[0])
                for h in range(2):
                    nc.vector.tensor_tensor(out=Q2[h][:], in0=Q2A[h][:],
                                            in1=Q2B[h][:], op=AL.max)
                    nc.scalar.activation(Q2[h][:], Q2[h][:], AF.Relu,
                                         scale=m2exp[:],
                                         accum_out=S2acc[:, h:h + 1])
                dump("d_Q2_0", Q2[0])
                dump("d_Q2_1", Q2[1])

            # ---- s2 + AllReduce + m3 ----
            s2ps = pp.tile([20, 2], fp32, tag="convps", name="convps")
            nc.tensor.matmul(s2ps[:], IND2b[:], S2acc[:], start=True, stop=True)
            s2loc = cpool.tile([20, 2], fp32, tag="s2loc", name="s2loc")
            nc.vector.tensor_copy(s2loc[:], s2ps[:])
            s2l = cpool.tile([20, 1], fp32, tag="s2l", name="s2l")
            nc.vector.tensor_tensor(out=s2l[:], in0=s2loc[:, 0:1],
                                    in1=s2loc[:, 1:2], op=AL.add)
            cc2_in = dpool.tile([20, 1], fp32)
            cc2_out = dpool.tile([20, 1], fp32)
            nc.sync.dma_start(cc2_in[:], s2l[:])
            nc.gpsimd.collective_compute(
                "AllReduce", AL.add, replica_groups=[list(range(N_CORES))],
                ins=[cc2_in.opt()], outs=[cc2_out.opt()])
            s2f = cpool.tile([20, 1], fp32, tag="s2f", name="s2f")
            nc.sync.dma_start(s2f[:], cc2_out[:])

            # ======================= LAYER 3 =======================
            with tc.tile_pool(name="l3big", bufs=1) as l3pool:
                # S3 valid-row staging (pads already zeroed up-front)
                for w in range(4):
                    y0, y1 = max(0, 2 * w - 2), min(8, 2 * w + 4)
                    iy0 = y0 - (2 * w - 2)
                    for ch in range(2):
                        ya, yb = max(y0, 4 * ch), min(y1, 4 * ch + 4)
                        if ya >= yb:
                            continue
                        la = 20 * (ya % 4)
                        da = 20 * (iy0 + (ya - y0))
                        nc.sync.dma_start(
                            S3[w][da:da + 20 * (yb - ya),
                                  2 * B_LOC:10 * B_LOC],
                            Q2[ch][la:la + 20 * (yb - ya), :])

                XP3 = [l3pool.tile([128, 4 * 256], bf16, tag=f"XP3_{t}", name=f"XP3_{t}")
                       for t in range(2)]
                tile_count = 0
                for t in range(2):
                    pss = [pp.tile([128, 512], fp32, tag="convps",
                                   name="convps") for _ in range(4)]
                    rhsvs = [S3[2 * t + c].rearrange("p (xp b) -> p b xp",
                                                     b=B_LOC)
                             for c in range(2)]
                    for srt in range(5):
                        for j in range(4):
                            for c in range(2):
                                nc.tensor.matmul(
                                    pss[j][64 * c:64 * c + 40, :],
                                    W3sb[:, srt * 40:(srt + 1) * 40],
                                    rhsvs[c][:, :, 2 * j + srt:2 * j + srt + 2],
                                    start=(srt == 0), stop=(srt == 4),
                                    tile_position=(0, 64 * c))
                    for j in range(4):
                        ps = pss[j]
                        use_act = (tile_count % 8) < int(ACT_EVAC_FRAC * 8)
                        tile_count += 1
                        if use_act:
                            sc = spool.tile([128, 512], bf16, tag="evac",
                                            name="evac")
                            nc.scalar.activation(sc[:], ps[:], AF.Copy)
                            vv = sc.rearrange("p (b x) -> p b x", x=2)
                            nc.vector.tensor_tensor(
                                out=XP3[t][:, j * 256:j * 256 + 256],
                                in0=vv[:, :, 0], in1=vv[:, :, 1], op=AL.max)
                        else:
                            nc.vector.tensor_reduce(
                                out=XP3[t][:, j * 256:j * 256 + 256],
                                in_=ps.rearrange("p (b x) -> p b x", x=2),
                                axis=AX.X, op=AL.max)

                R3A = l3pool.tile([80, 4 * 256], bf16, tag="R3A", name="R3A")
                R3B = l3pool.tile([80, 4 * 256], bf16, tag="R3B", name="R3B")
                # R3/P3 lanes: w*20 + oc
                _dq = 0
                for t in range(2):
                    for c in range(2):
                        w = 2 * t + c
                        for par, dstt in ((0, R3A), (1, R3B)):
                            eng = nc.sync if _dq % 2 == 0 else nc.gpsimd
                            _dq += 1
                            eng.dma_start(
                                dstt[20 * w:20 * w + 20, :],
                                XP3[t][64 * c + 20 * par:
                                       64 * c + 20 * par + 20, :])
                R3 = l3pool.tile([80, 4 * 256], bf16, tag="R3", name="R3")
                nc.vector.tensor_tensor(out=R3[:], in0=R3A[:], in1=R3B[:],
                                        op=AL.max)
                nc.scalar.activation(P3[:], R3[:], AF.Relu)
                dump("d_S3_1", S3[1])
                dump("d_XP3_0", XP3[0])
                dump("d_P3", P3)

            v3 = cpool.tile([20, 1], fp32, tag="v3", name="v3")
            nc.vector.tensor_tensor(out=v3[:], in0=s2f[:], in1=m2col[:],
                                    op=AL.mult)
            m3col = hash_chain(v3, A3Tb, BW3b, CW3b, ONES3b, 30, 3, "m3")
            m3ps = pp.tile([80, 1], fp32, tag="convps", name="convps")
            nc.tensor.matmul(m3ps[:], EXP2b[:], m3col[:], start=True, stop=True)
            m3exp = cpool.tile([80, 1], fp32, tag="m3exp", name="m3exp")
            nc.vector.tensor_copy(m3exp[:], m3ps[:])

            # ======================= FC =======================
            WoutM = cpool.tile([80, 40], bf16, tag="WoutM", name="WoutM")
            nc.vector.tensor_scalar(out=WoutM[:], in0=Woutsb[:],
                                    scalar1=m3exp[:], scalar2=None,
                                    op0=AL.mult)
            fcps = pp.tile([10, 256], fp32, tag="convps", name="convps")
            for xq in range(4):
                nc.tensor.matmul(fcps[:],
                                 WoutM[:, xq * 10:(xq + 1) * 10],
                                 P3[:, xq * 256:(xq + 1) * 256],
                                 start=(xq == 0), stop=(xq == 3))
            if debug:
                dm = dbgpool.tile([80, 4], fp32, tag="dumpm", name="dumpm")
                nc.vector.tensor_copy(dm[:, 0:1], m2exp[:])
                nc.vector.tensor_copy(dm[:, 1:2], m3exp[:])
                nc.vector.tensor_copy(dm[0:16, 2:3], s1f[:])
                nc.vector.tensor_copy(dm[0:20, 3:4], s2f[:])
                nc.sync.dma_start(dbg["d_m"].ap(), dm[:])
            out_sb = cpool.tile([10, 256], fp32, tag="out_sb", name="out_sb")
            nc.scalar.activation(out_sb[:], fcps[:], AF.Identity, bias=boutb[:])
            nc.sync.dma_start(out_ext.ap(), out_sb[:])

    nc.compile()
    return nc


_NC_CACHE = None


def kernel(**inputs) -> np.ndarray:
    return _run(inputs)[0]


def _run(inputs, force_masks=False, debug=False):
    global _NC_CACHE
    shared, xs_all = _host_prep(**inputs, force_masks=force_masks)
    if _NC_CACHE is None:
        _NC_CACHE = build_nc(debug=debug)
    nc = _NC_CACHE
    in_maps = []
    for core in range(N_CORES):
        m = dict(shared)
        m["X4"] = xs_all[core]
        in_maps.append(m)
    res = run_bass_kernel_spmd(nc, in_maps, core_ids=list(range(N_CORES)))
    outs = [res.results[i]["out"] for i in range(N_CORES)]  # [10, 256] each
    full = np.concatenate([o.T for o in outs], axis=0)      # (2048, 10)
    return full.astype(np.float32), res.results


if __name__ == "__main__":
    import reference as R
    inputs = {k: np.asarray(v) for k, v in R.setup_inputs().items()}
    out = kernel(**inputs)
    print("kernel out", out.shape, "absmax", np.abs(out).max())

